# revision 33
# baseline (speedup 1.0000x reference)
"""MoE dispatch (DispatchSF) Trainium2 Bass kernel — expert-parallel over 8 cores.

Problem: N=4096 tokens, D=1024 d_model, E=8 experts. For each expert e:
pack tokens with hot_mask[:, e] > 0 (in original order) into the first
`count` of N output slots, scaled by score[:, e]; zero-pad the rest.
One expert per NeuronCore; each core sees the full token buffer.

Device algorithm per core:
  1. exclusive prefix-sum of the mask over token order, computed directly in
     the DMA-ucode "wrapped" index layout [16, 256] (token i at [i%16, i//16]):
     within-column via a [16,16] strict-upper-triangular PE matmul, across
     columns via a [1, 256] tensor_tensor_scan + K=1 broadcast matmul.
  2. compaction: ONE dma_scatter_add ucode op scatters (token_as_f32, score)
     pairs into a 256B-strided [2N, 64] f32 output buffer (opair) at row
     `excl_prefix` for selected tokens, row `N + token` (dump) otherwise.
     opair rows 0..N-1 are pre-zeroed, so add == write and the packed zone's
     tail stays (0.0, 0.0).
  3. rebuild the packed token list as a wrapped int16 index table
     (load-back + two PE transposes), pad it with -1 beyond
     ceil(count/128)*128 slots (chunk heads kept valid).
  4. 4 chunked dma_gather ucode ops fetch the selected rows of xcat
     (= x row ++ score ++ pad, 1088 f32) — only ~count rows are read.
     Per 128-slot block: DVE scale by (score lane x slot-validity), then a
     conditional store; blocks past `count` are skipped entirely and stay
     zero via the donated zero output buffer.

Host slices out_tags from opair[:, :N, 0] and counts from ocnt.
"""

import os
import sys

import numpy as np

N, D, E = 4096, 1024, 8
P = 128
F = N // P          # 32 blocks of 128 slots
W = N // 16         # 256 wrapped columns
DC = D + 64         # concat row: x ++ score ++ pad (4352 B, 17*256)
NCHUNK = 8
CH = N // NCHUNK    # 1024 slots per gather chunk
PAIR_STRIDE = 64    # f32 elements per opair row (256 B stride requirement)

_CACHE = {}


def _ensure_path():
    for p in ("/opt/trn_rl_repo",):
        if p not in sys.path:
            sys.path.insert(0, p)


def _emit(tc, nc, ins, outs):
    """Emit the per-core device program. ins/outs: dicts of DRAM APs."""
    import concourse.mybir as mybir
    from concourse.masks import make_identity, make_upper_triangular

    f32 = mybir.dt.float32
    i32 = mybir.dt.int32
    i16 = mybir.dt.int16
    AO = mybir.AluOpType

    xcat = ins["xcat"]    # [N, DC] f32: row = x[i] ++ score[i] ++ zeros
    m16 = ins["m16"]      # [16, W] i32 wrapped mask: [q, s] = mask[s*16+q]
    scol = ins["scol"]    # [P, F] f32: [p, f] = score[f*128 + p]
    odata = outs["odata"]  # [N, D] f32
    opair = outs["opair"]  # [2N, 64] f32; row s cols 0:2 = (token, score)
    ocnt = outs["ocnt"]    # [1, 1] i32

    with (
        tc.tile_pool(name="small", bufs=1) as sp,
        tc.tile_pool(name="psum", bufs=1, space="PSUM") as pp,
        tc.tile_pool(name="xc", bufs=NCHUNK) as xcp,
    ):
        # ---------- constants ----------
        utri16 = sp.tile([16, 16], f32)
        make_upper_triangular(nc, utri16[:], val=1.0, diag=False)
        ident = sp.tile([P, P], f32)
        make_identity(nc, ident[:])
        ones16c = sp.tile([16, 1], f32)
        nc.vector.memset(ones16c[:], 1.0)
        ones16r = sp.tile([1, 16], f32)
        nc.vector.memset(ones16r[:], 1.0)
        ones128r = sp.tile([1, P], f32)
        nc.vector.memset(ones128r[:], 1.0)

        tok16I = sp.tile([16, W], i32)   # token id q + 16s
        nc.gpsimd.iota(tok16I[:], pattern=[[16, W]], base=0, channel_multiplier=1)
        tok16F = sp.tile([16, W], f32)
        nc.vector.tensor_copy(tok16F[:], tok16I[:])
        tok128I = sp.tile([P, F], i32)   # token/slot id p + 128f
        nc.gpsimd.iota(tok128I[:], pattern=[[P, F]], base=0, channel_multiplier=1)
        tok128F = sp.tile([P, F], f32)
        nc.vector.tensor_copy(tok128F[:], tok128I[:])
        blkI = sp.tile([1, F], i32)      # 128*j block starts
        nc.gpsimd.iota(blkI[:], pattern=[[P, F]], base=0, channel_multiplier=0)
        blkF = sp.tile([1, F], f32)
        nc.vector.tensor_copy(blkF[:], blkI[:])
        chI = sp.tile([1, NCHUNK], i32)  # 1024*c chunk starts
        nc.gpsimd.iota(chI[:], pattern=[[CH, NCHUNK]], base=0, channel_multiplier=0)
        chF = sp.tile([1, NCHUNK], f32)
        nc.vector.tensor_copy(chF[:], chI[:])

        # ---------- exclusive prefix over token order (wrapped layout) ----------
        m16I = sp.tile([16, W], i32)
        nc.sync.dma_start(m16I[:], m16)
        m16F = sp.tile([16, W], f32)
        nc.vector.tensor_copy(m16F[:], m16I[:])

        excl_ps = pp.tile([16, W], f32, space="PSUM")
        nc.tensor.matmul(excl_ps[:], lhsT=utri16[:], rhs=m16F[:],
                         start=True, stop=True)
        colsum_ps = pp.tile([1, W], f32, space="PSUM")
        nc.tensor.matmul(colsum_ps[:], lhsT=ones16c[:], rhs=m16F[:],
                         start=True, stop=True)
        # scan directly off PSUM (data1 is ignored under bypass but must be
        # a non-PSUM operand; hm is a ready zero-filled SBUF tile)
        hm = sp.tile([1, W], f32)
        nc.vector.memset(hm[:], 0.0)
        for c in range(NCHUNK):
            nc.vector.memset(hm[:, (CH // 16) * c:(CH // 16) * c + 1], 1.0)
        colsumS = sp.tile([1, W], f32)
        nc.vector.tensor_copy(colsumS[:], colsum_ps[:])
        colincl = sp.tile([1, W], f32)
        nc.vector.tensor_tensor_scan(
            colincl[:], colsumS[:], hm[:], initial=0.0,
            op0=AO.add, op1=AO.bypass,
        )
        colexcl = sp.tile([1, W], f32)
        nc.vector.tensor_tensor(colexcl[:], colincl[:], colsumS[:],
                                op=AO.subtract)
        coloff_ps = pp.tile([16, W], f32, space="PSUM")
        nc.tensor.matmul(coloff_ps[:], lhsT=ones16r[:], rhs=colexcl[:],
                         start=True, stop=True)
        coloffS = sp.tile([16, W], f32)
        nc.vector.tensor_copy(coloffS[:], coloff_ps[:])
        excl16 = sp.tile([16, W], f32)
        nc.vector.tensor_tensor(excl16[:], excl_ps[:], coloffS[:], op=AO.add)

        # count
        cntF = sp.tile([1, 1], f32)
        nc.vector.tensor_copy(cntF[:], colincl[:, W - 1:W])
        cntI = sp.tile([1, 1], i32)
        nc.vector.tensor_copy(cntI[:], cntF[:])
        nc.sync.dma_start(ocnt[:], cntI[:])

        # scatter dest = m ? excl : N + token  (unique rows in [0, 2N))
        # d1 = (excl - N) - tok ; d2 = d1 * m ; dest = (d2 + N) + tok
        d1 = sp.tile([16, W], f32)
        nc.vector.scalar_tensor_tensor(d1[:], excl16[:], -float(N), tok16F[:],
                                       op0=AO.add, op1=AO.subtract)
        d2 = sp.tile([16, W], f32)
        nc.vector.tensor_tensor(d2[:], d1[:], m16F[:], op=AO.mult)
        destW = sp.tile([16, W], f32)
        nc.vector.scalar_tensor_tensor(destW[:], d2[:], float(N), tok16F[:],
                                       op0=AO.add, op1=AO.add)

        # wrapped scatter idx table, int16, replicated to 128 partitions
        idx16 = sp.tile([P, W], i16)
        nc.vector.tensor_copy(idx16[0:16, :], destW[:])
        nc.sync.dma_start(idx16[16:32, :], idx16[0:16, :])
        nc.vector.tensor_copy(idx16[32:64, :], idx16[0:32, :])
        nc.vector.tensor_copy(idx16[64:128, :], idx16[0:64, :])

        # ---------- compaction scatter ----------
        # payload (token, score) for token i at [i%128, i//128]
        sF = sp.tile([P, F], f32)
        nc.sync.dma_start(sF[:], scol)
        pairV = sp.tile([P, 2 * F], f32)
        pair2 = pairV[:].rearrange("p (f c) -> p f c", c=2)
        nc.vector.tensor_copy(pair2[:, :, 0], tok128F[:])
        nc.vector.tensor_copy(pair2[:, :, 1], sF[:])

        # pre-zero packed zone (rows 0..N-1): one contiguous 1 MiB write
        zpair = sp.tile([P, N * PAIR_STRIDE // P], f32)
        nc.vector.memset(zpair[:], 0.0)
        nc.sync.dma_start(
            opair[0:N, :].rearrange("(p t) c -> p (t c)", p=P), zpair[:],
        )

        nc.gpsimd.dma_scatter_add(
            out_ap=opair[:, 0:2],
            in_ap=pair2[:, :, :],
            idxs_ap=idx16[:],
            num_idxs=N,
            num_idxs_reg=N,
            elem_size=2,
            elem_step=PAIR_STRIDE,
        )

        # ---------- rebuild packed tokens as wrapped gather idx ----------
        # load the whole packed zone contiguously (1 MiB, 128 fat descriptors);
        # tokens of slot s = 32p + t sit at ldFull[p, 64t]
        ldFull = sp.tile([P, N * PAIR_STRIDE // P], f32)
        nc.sync.dma_start(
            ldFull[:], opair[0:N, :].rearrange("(p t) c -> p (t c)", p=P),
        )
        ld3 = ldFull[:].rearrange("p (t c) -> p t c", c=PAIR_STRIDE)
        ldT = sp.tile([P, F], f32)
        nc.vector.tensor_copy(ldT[:], ld3[:, :, 0])
        # wrapped[q, 2p+u] = token of slot 32p + 16u + q -> two PE transposes
        t0_ps = pp.tile([16, P], f32, space="PSUM")
        nc.tensor.transpose(t0_ps[:], ldT[:, 0:16], ident[:])
        t1_ps = pp.tile([16, P], f32, space="PSUM")
        nc.tensor.transpose(t1_ps[:], ldT[:, 16:32], ident[:])

        # block validity bv[j] = (128j < count), expanded to wrapped columns
        bv = sp.tile([1, F], f32)
        nc.vector.tensor_scalar(bv[:], blkF[:], scalar1=cntF[:, :1],
                                scalar2=None, op0=AO.is_lt)
        bv256 = sp.tile([1, W], f32)
        bv3 = bv256[:].rearrange("p (a k) -> p a k", k=8)
        nc.vector.tensor_copy(bv3[:, :, :],
                              bv[:, :, None].broadcast_to([1, F, 8]))
        # hm (chunk-head columns = 1) keeps every dma_gather chunk non-empty
        bvh = sp.tile([1, W], f32)
        nc.vector.tensor_tensor(bvh[:], bv256[:], hm[:], op=AO.max)
        bvh16_ps = pp.tile([16, W], f32, space="PSUM")
        nc.tensor.matmul(bvh16_ps[:], lhsT=ones16r[:], rhs=bvh[:],
                         start=True, stop=True)

        # idw[q, s] = packed token of slot 16s+q; adj = (idw+1)*bvh - 1
        idw = sp.tile([16, W], f32)
        idw3 = idw[:].rearrange("q (p u) -> q p u", u=2)
        nc.vector.tensor_copy(idw3[:, :, 0], t0_ps[:])
        nc.vector.tensor_copy(idw3[:, :, 1], t1_ps[:])
        adjF = sp.tile([16, W], f32)
        nc.vector.scalar_tensor_tensor(adjF[:], idw[:], 1.0, bvh16_ps[:],
                                       op0=AO.add, op1=AO.mult)
        idxg = sp.tile([P, W], i16)
        nc.vector.tensor_scalar(idxg[0:16, :], adjF[:], scalar1=1.0,
                                scalar2=None, op0=AO.subtract)
        nc.sync.dma_start(idxg[16:32, :], idxg[0:16, :])
        nc.vector.tensor_copy(idxg[32:64, :], idxg[0:32, :])
        nc.vector.tensor_copy(idxg[64:128, :], idxg[0:64, :])

        # ---------- runtime registers ----------
        # per-chunk gather counts r_c = clamp(128*sum(bv) - 1024c, 1, 1024)
        nb = sp.tile([1, 1], f32)
        nc.vector.tensor_reduce(nb[:], bv[:], axis=mybir.AxisListType.X,
                                op=AO.add)
        cnt128F = sp.tile([1, 1], f32)
        nc.vector.tensor_scalar(cnt128F[:], nb[:], scalar1=float(P),
                                scalar2=None, op0=AO.mult)
        r4 = sp.tile([1, NCHUNK], f32)
        nc.vector.tensor_scalar(r4[:], chF[:], scalar1=cnt128F[:, :1],
                                scalar2=-1.0, op0=AO.subtract, op1=AO.mult)
        nc.vector.tensor_scalar(r4[:], r4[:], scalar1=16.0, scalar2=float(CH),
                                op0=AO.max, op1=AO.min)
        r4I = sp.tile([1, NCHUNK], i32)
        nc.vector.tensor_copy(r4I[:], r4[:])
        _, r_vals = nc.values_load_multi_w_load_instructions(
            r4I[:], min_val=1, max_val=CH, skip_runtime_bounds_check=True)
        cnt_rv = nc.values_load(cntI[:], min_val=0, max_val=N,
                                skip_runtime_bounds_check=True)

        # slot validity for the straddle block: valid[p, f] = (p + 128f < count)
        cntB_ps = pp.tile([P, 1], f32, space="PSUM")
        nc.tensor.matmul(cntB_ps[:], lhsT=ones128r[:], rhs=cntF[:],
                         start=True, stop=True)
        cntBS = sp.tile([P, 1], f32)
        nc.vector.tensor_copy(cntBS[:], cntB_ps[:])
        validF = sp.tile([P, F], f32)
        nc.vector.tensor_scalar(validF[:], tok128F[:], scalar1=cntBS[:, :1],
                                scalar2=None, op0=AO.is_lt)

        # ---------- gather chunks, scale, conditional store ----------
        FB = F // NCHUNK  # blocks per chunk (8)
        for c in range(NCHUNK):
            xg = xcp.tile([P, FB, DC], f32, tag="xc")
            nc.gpsimd.dma_gather(
                xg[:, :, :],
                xcat,
                idxg[:, (CH // 16) * c:(CH // 16) * (c + 1)],
                num_idxs=CH,
                num_idxs_reg=r_vals[c],
                elem_size=DC,
            )
            smul = sp.tile([P, FB], f32, tag="smul")
            nc.vector.tensor_tensor(smul[:], xg[:, :, D],
                                    validF[:, FB * c:FB * (c + 1)], op=AO.mult)
            for j in range(FB):
                jj = FB * c + j
                nc.vector.tensor_scalar(
                    xg[:, j, 0:D], xg[:, j, 0:D], scalar1=smul[:, j:j + 1],
                    scalar2=None, op0=AO.mult,
                )
                nc.sync.dma_start(
                    odata[P * jj:P * (jj + 1), :], xg[:, j, 0:D],
                    cond=cnt_rv > P * jj,
                )


def _build():
    _ensure_path()
    import concourse.bacc as bacc
    import concourse.mybir as mybir
    import concourse.tile as tile

    f32 = mybir.dt.float32
    i32 = mybir.dt.int32

    nc = bacc.Bacc(
        "TRN2",
        target_bir_lowering=False,
        debug=False,
        enable_asserts=True,
        num_devices=E,
    )
    ins = {
        "xcat": nc.dram_tensor("xcat", [N, DC], f32, kind="ExternalInput").ap(),
        "m16": nc.dram_tensor("m16", [16, W], i32, kind="ExternalInput").ap(),
        "scol": nc.dram_tensor("scol", [P, F], f32, kind="ExternalInput").ap(),
    }
    outs = {
        "odata": nc.dram_tensor("odata", [N, D], f32, kind="ExternalOutput").ap(),
        "opair": nc.dram_tensor("opair", [2 * N, PAIR_STRIDE], f32,
                                kind="ExternalOutput").ap(),
        "ocnt": nc.dram_tensor("ocnt", [1, 1], i32, kind="ExternalOutput").ap(),
    }
    with tile.TileContext(nc) as tc:
        _emit(tc, nc, ins, outs)
    nc.compile()
    return nc


def _get_nc():
    if "nc" not in _CACHE:
        _CACHE["nc"] = _build()
    return _CACHE["nc"]


def _install_ntff_hook():
    """Provide antenv.axon_hooks if the image lacks it (enables trace=True)."""
    try:
        from antenv.axon_hooks import get_axon_ntff_profile_hook  # noqa: F401
        return
    except ImportError:
        pass
    try:
        import types

        import antenv
        from trn_agent_boot.trn_boot import _ntff_profile_via_ctypes

        hook = _ntff_profile_via_ctypes("/opt/axon/libaxon_pjrt.so")
        mod = types.ModuleType("antenv.axon_hooks")
        mod.get_axon_ntff_profile_hook = lambda: hook
        mod.set_axon_ntff_profile_hook = lambda h: None
        sys.modules["antenv.axon_hooks"] = mod
        antenv.axon_hooks = mod
    except Exception:
        pass


def kernel(x, score, hot_mask, tag):
    _ensure_path()
    _install_ntff_hook()
    from concourse.bass_utils import run_bass_kernel_spmd

    x = np.ascontiguousarray(np.asarray(x, dtype=np.float32))
    score = np.asarray(score, dtype=np.float32)
    hot_mask = np.asarray(hot_mask, dtype=np.int32)

    nc = _get_nc()
    in_maps = []
    for e in range(E):
        xcat = np.zeros((N, DC), dtype=np.float32)
        xcat[:, :D] = x
        xcat[:, D] = score[:, e]
        in_maps.append({
            "xcat": xcat,
            "m16": np.ascontiguousarray(hot_mask[:, e].reshape(W, 16).T),
            "scol": np.ascontiguousarray(score[:, e].reshape(F, P).T),
        })
    trace = bool(int(os.environ.get("KERNEL_TRACE", "0")))
    res = run_bass_kernel_spmd(nc, in_maps, core_ids=list(range(E)), trace=trace)
    _CACHE["last_results"] = res

    out_data = np.stack([res.results[e]["odata"] for e in range(E)])
    out_tags = np.stack([res.results[e]["opair"][:N, 0:1].astype(np.int32)
                         for e in range(E)])
    counts = np.array([res.results[e]["ocnt"][0, 0] for e in range(E)],
                      dtype=np.int32)
    return out_data, out_tags, counts


# revision 34
# speedup vs baseline: 1.0925x; 1.0925x over previous
"""MoE dispatch (DispatchSF) Trainium2 Bass kernel — expert-parallel over 8 cores.

Problem: N=4096 tokens, D=1024 d_model, E=8 experts. For each expert e:
pack tokens with hot_mask[:, e] > 0 (in original order) into the first
`count` of N output slots, scaled by score[:, e]; zero-pad the rest.
One expert per NeuronCore; each core sees the full token buffer.

Device algorithm per core:
  1. exclusive prefix-sum of the mask over token order, computed directly in
     the DMA-ucode "wrapped" index layout [16, 256] (token i at [i%16, i//16]):
     within-column via a [16,16] strict-upper-triangular PE matmul, across
     columns via a [1, 256] tensor_tensor_scan + K=1 broadcast matmul.
  2. compaction: ONE dma_scatter_add ucode op scatters (token_as_f32, score)
     pairs into a 256B-strided [2N, 64] f32 output buffer (opair) at row
     `excl_prefix` for selected tokens, row `N + token` (dump) otherwise.
     opair rows 0..N-1 are pre-zeroed, so add == write and the packed zone's
     tail stays (0.0, 0.0).
  3. rebuild the packed token list as a wrapped int16 index table
     (load-back + two PE transposes), pad it with -1 beyond
     ceil(count/128)*128 slots (chunk heads kept valid).
  4. 4 chunked dma_gather ucode ops fetch the selected rows of xcat
     (= x row ++ score ++ pad, 1088 f32) — only ~count rows are read.
     Per 128-slot block: DVE scale by (score lane x slot-validity), then a
     conditional store; blocks past `count` are skipped entirely and stay
     zero via the donated zero output buffer.

Host slices out_tags from opair[:, :N, 0] and counts from ocnt.
"""

import os
import sys

import numpy as np

N, D, E = 4096, 1024, 8
P = 128
F = N // P          # 32 blocks of 128 slots
W = N // 16         # 256 wrapped columns
DC = D + 64         # concat row: x ++ score ++ pad (4352 B, 17*256)
NCHUNK = 4
CH = N // NCHUNK    # 1024 slots per gather chunk
PAIR_STRIDE = 64    # f32 elements per opair row (256 B stride requirement)

_CACHE = {}


def _ensure_path():
    for p in ("/opt/trn_rl_repo",):
        if p not in sys.path:
            sys.path.insert(0, p)


def _emit(tc, nc, ins, outs):
    """Emit the per-core device program. ins/outs: dicts of DRAM APs."""
    import concourse.mybir as mybir
    from concourse.masks import make_identity, make_upper_triangular

    f32 = mybir.dt.float32
    i32 = mybir.dt.int32
    i16 = mybir.dt.int16
    AO = mybir.AluOpType

    xcat = ins["xcat"]    # [N, DC] f32: row = x[i] ++ score[i] ++ zeros
    m16 = ins["m16"]      # [16, W] i32 wrapped mask: [q, s] = mask[s*16+q]
    scol = ins["scol"]    # [P, F] f32: [p, f] = score[f*128 + p]
    odata = outs["odata"]  # [N, D] f32
    opair = outs["opair"]  # [2N, 64] f32; row s cols 0:2 = (token, score)
    ocnt = outs["ocnt"]    # [1, 1] i32

    with (
        tc.tile_pool(name="small", bufs=1) as sp,
        tc.tile_pool(name="psum", bufs=1, space="PSUM") as pp,
        tc.tile_pool(name="xc", bufs=NCHUNK) as xcp,
    ):
        # ---------- constants ----------
        utri16 = sp.tile([16, 16], f32)
        make_upper_triangular(nc, utri16[:], val=1.0, diag=False)
        ident = sp.tile([P, P], f32)
        make_identity(nc, ident[:])
        ones16c = sp.tile([16, 1], f32)
        nc.vector.memset(ones16c[:], 1.0)
        ones16r = sp.tile([1, 16], f32)
        nc.vector.memset(ones16r[:], 1.0)
        ones128r = sp.tile([1, P], f32)
        nc.vector.memset(ones128r[:], 1.0)

        tok16I = sp.tile([16, W], i32)   # token id q + 16s
        nc.gpsimd.iota(tok16I[:], pattern=[[16, W]], base=0, channel_multiplier=1)
        tok16F = sp.tile([16, W], f32)
        nc.vector.tensor_copy(tok16F[:], tok16I[:])
        tok128I = sp.tile([P, F], i32)   # token/slot id p + 128f
        nc.gpsimd.iota(tok128I[:], pattern=[[P, F]], base=0, channel_multiplier=1)
        tok128F = sp.tile([P, F], f32)
        nc.vector.tensor_copy(tok128F[:], tok128I[:])
        blkI = sp.tile([1, F], i32)      # 128*j block starts
        nc.gpsimd.iota(blkI[:], pattern=[[P, F]], base=0, channel_multiplier=0)
        blkF = sp.tile([1, F], f32)
        nc.vector.tensor_copy(blkF[:], blkI[:])
        chI = sp.tile([1, NCHUNK], i32)  # 1024*c chunk starts
        nc.gpsimd.iota(chI[:], pattern=[[CH, NCHUNK]], base=0, channel_multiplier=0)
        chF = sp.tile([1, NCHUNK], f32)
        nc.vector.tensor_copy(chF[:], chI[:])

        # ---------- exclusive prefix over token order (wrapped layout) ----------
        m16I = sp.tile([16, W], i32)
        nc.sync.dma_start(m16I[:], m16)
        m16F = sp.tile([16, W], f32)
        nc.vector.tensor_copy(m16F[:], m16I[:])

        excl_ps = pp.tile([16, W], f32, space="PSUM")
        nc.tensor.matmul(excl_ps[:], lhsT=utri16[:], rhs=m16F[:],
                         start=True, stop=True)
        colsum_ps = pp.tile([1, W], f32, space="PSUM")
        nc.tensor.matmul(colsum_ps[:], lhsT=ones16c[:], rhs=m16F[:],
                         start=True, stop=True)
        # scan directly off PSUM (data1 is ignored under bypass but must be
        # a non-PSUM operand; hm is a ready zero-filled SBUF tile)
        hm = sp.tile([1, W], f32)
        nc.vector.memset(hm[:], 0.0)
        for c in range(NCHUNK):
            nc.vector.memset(hm[:, (CH // 16) * c:(CH // 16) * c + 1], 1.0)
        colsumS = sp.tile([1, W], f32)
        nc.vector.tensor_copy(colsumS[:], colsum_ps[:])
        colincl = sp.tile([1, W], f32)
        nc.vector.tensor_tensor_scan(
            colincl[:], colsumS[:], hm[:], initial=0.0,
            op0=AO.add, op1=AO.bypass,
        )
        colexcl = sp.tile([1, W], f32)
        nc.vector.tensor_tensor(colexcl[:], colincl[:], colsumS[:],
                                op=AO.subtract)
        coloff_ps = pp.tile([16, W], f32, space="PSUM")
        nc.tensor.matmul(coloff_ps[:], lhsT=ones16r[:], rhs=colexcl[:],
                         start=True, stop=True)
        coloffS = sp.tile([16, W], f32)
        nc.vector.tensor_copy(coloffS[:], coloff_ps[:])
        excl16 = sp.tile([16, W], f32)
        nc.vector.tensor_tensor(excl16[:], excl_ps[:], coloffS[:], op=AO.add)

        # count
        cntF = sp.tile([1, 1], f32)
        nc.vector.tensor_copy(cntF[:], colincl[:, W - 1:W])
        cntI = sp.tile([1, 1], i32)
        nc.vector.tensor_copy(cntI[:], cntF[:])
        nc.sync.dma_start(ocnt[:], cntI[:])

        # scatter dest = m ? excl : N + token  (unique rows in [0, 2N))
        # d1 = (excl - N) - tok ; d2 = d1 * m ; dest = (d2 + N) + tok
        d1 = sp.tile([16, W], f32)
        nc.vector.scalar_tensor_tensor(d1[:], excl16[:], -float(N), tok16F[:],
                                       op0=AO.add, op1=AO.subtract)
        d2 = sp.tile([16, W], f32)
        nc.vector.tensor_tensor(d2[:], d1[:], m16F[:], op=AO.mult)
        destW = sp.tile([16, W], f32)
        nc.vector.scalar_tensor_tensor(destW[:], d2[:], float(N), tok16F[:],
                                       op0=AO.add, op1=AO.add)

        # wrapped scatter idx table, int16, replicated to 128 partitions
        idx16 = sp.tile([P, W], i16)
        nc.vector.tensor_copy(idx16[0:16, :], destW[:])
        nc.sync.dma_start(idx16[16:32, :], idx16[0:16, :])
        nc.vector.tensor_copy(idx16[32:64, :], idx16[0:32, :])
        nc.vector.tensor_copy(idx16[64:128, :], idx16[0:64, :])

        # ---------- compaction scatter ----------
        # payload (token, score) for token i at [i%128, i//128]
        sF = sp.tile([P, F], f32)
        nc.sync.dma_start(sF[:], scol)
        pairV = sp.tile([P, 2 * F], f32)
        pair2 = pairV[:].rearrange("p (f c) -> p f c", c=2)
        nc.vector.tensor_copy(pair2[:, :, 0], tok128F[:])
        nc.vector.tensor_copy(pair2[:, :, 1], sF[:])

        # pre-zero packed zone (rows 0..N-1): one contiguous 1 MiB write
        zpair = sp.tile([P, N * PAIR_STRIDE // P], f32)
        nc.vector.memset(zpair[:], 0.0)
        nc.sync.dma_start(
            opair[0:N, :].rearrange("(p t) c -> p (t c)", p=P), zpair[:],
        )

        nc.gpsimd.dma_scatter_add(
            out_ap=opair[:, 0:2],
            in_ap=pair2[:, :, :],
            idxs_ap=idx16[:],
            num_idxs=N,
            num_idxs_reg=N,
            elem_size=2,
            elem_step=PAIR_STRIDE,
            single_packet=False,
        )

        # ---------- rebuild packed tokens as wrapped gather idx ----------
        # load the whole packed zone contiguously (1 MiB, 128 fat descriptors);
        # tokens of slot s = 32p + t sit at ldFull[p, 64t]
        ldFull = sp.tile([P, N * PAIR_STRIDE // P], f32)
        nc.sync.dma_start(
            ldFull[:], opair[0:N, :].rearrange("(p t) c -> p (t c)", p=P),
        )
        ld3 = ldFull[:].rearrange("p (t c) -> p t c", c=PAIR_STRIDE)
        ldT = sp.tile([P, F], f32)
        nc.vector.tensor_copy(ldT[:], ld3[:, :, 0])
        # wrapped[q, 2p+u] = token of slot 32p + 16u + q -> two PE transposes
        t0_ps = pp.tile([16, P], f32, space="PSUM")
        nc.tensor.transpose(t0_ps[:], ldT[:, 0:16], ident[:])
        t1_ps = pp.tile([16, P], f32, space="PSUM")
        nc.tensor.transpose(t1_ps[:], ldT[:, 16:32], ident[:])

        # block validity bv[j] = (128j < count), expanded to wrapped columns
        bv = sp.tile([1, F], f32)
        nc.vector.tensor_scalar(bv[:], blkF[:], scalar1=cntF[:, :1],
                                scalar2=None, op0=AO.is_lt)
        bv256 = sp.tile([1, W], f32)
        bv3 = bv256[:].rearrange("p (a k) -> p a k", k=8)
        nc.vector.tensor_copy(bv3[:, :, :],
                              bv[:, :, None].broadcast_to([1, F, 8]))
        # hm (chunk-head columns = 1) keeps every dma_gather chunk non-empty
        bvh = sp.tile([1, W], f32)
        nc.vector.tensor_tensor(bvh[:], bv256[:], hm[:], op=AO.max)
        bvh16_ps = pp.tile([16, W], f32, space="PSUM")
        nc.tensor.matmul(bvh16_ps[:], lhsT=ones16r[:], rhs=bvh[:],
                         start=True, stop=True)

        # idw[q, s] = packed token of slot 16s+q; adj = (idw+1)*bvh - 1
        idw = sp.tile([16, W], f32)
        idw3 = idw[:].rearrange("q (p u) -> q p u", u=2)
        nc.vector.tensor_copy(idw3[:, :, 0], t0_ps[:])
        nc.vector.tensor_copy(idw3[:, :, 1], t1_ps[:])
        adjF = sp.tile([16, W], f32)
        nc.vector.scalar_tensor_tensor(adjF[:], idw[:], 1.0, bvh16_ps[:],
                                       op0=AO.add, op1=AO.mult)
        idxg = sp.tile([P, W], i16)
        nc.vector.tensor_scalar(idxg[0:16, :], adjF[:], scalar1=1.0,
                                scalar2=None, op0=AO.subtract)
        nc.sync.dma_start(idxg[16:32, :], idxg[0:16, :])
        nc.vector.tensor_copy(idxg[32:64, :], idxg[0:32, :])
        nc.vector.tensor_copy(idxg[64:128, :], idxg[0:64, :])

        # ---------- runtime registers ----------
        # per-chunk gather counts r_c = clamp(128*sum(bv) - 1024c, 1, 1024)
        nb = sp.tile([1, 1], f32)
        nc.vector.tensor_reduce(nb[:], bv[:], axis=mybir.AxisListType.X,
                                op=AO.add)
        cnt128F = sp.tile([1, 1], f32)
        nc.vector.tensor_scalar(cnt128F[:], nb[:], scalar1=float(P),
                                scalar2=None, op0=AO.mult)
        r4 = sp.tile([1, NCHUNK], f32)
        nc.vector.tensor_scalar(r4[:], chF[:], scalar1=cnt128F[:, :1],
                                scalar2=-1.0, op0=AO.subtract, op1=AO.mult)
        nc.vector.tensor_scalar(r4[:], r4[:], scalar1=16.0, scalar2=float(CH),
                                op0=AO.max, op1=AO.min)
        r4I = sp.tile([1, NCHUNK], i32)
        nc.vector.tensor_copy(r4I[:], r4[:])
        _, r_vals = nc.values_load_multi_w_load_instructions(
            r4I[:], min_val=1, max_val=CH, skip_runtime_bounds_check=True)
        cnt_rv = nc.values_load(cntI[:], min_val=0, max_val=N,
                                skip_runtime_bounds_check=True)

        # slot validity for the straddle block: valid[p, f] = (p + 128f < count)
        cntB_ps = pp.tile([P, 1], f32, space="PSUM")
        nc.tensor.matmul(cntB_ps[:], lhsT=ones128r[:], rhs=cntF[:],
                         start=True, stop=True)
        cntBS = sp.tile([P, 1], f32)
        nc.vector.tensor_copy(cntBS[:], cntB_ps[:])
        validF = sp.tile([P, F], f32)
        nc.vector.tensor_scalar(validF[:], tok128F[:], scalar1=cntBS[:, :1],
                                scalar2=None, op0=AO.is_lt)

        # ---------- gather chunks, scale, conditional store ----------
        FB = F // NCHUNK  # blocks per chunk (8)
        for c in range(NCHUNK):
            xg = xcp.tile([P, FB, DC], f32, tag="xc")
            nc.gpsimd.dma_gather(
                xg[:, :, :],
                xcat,
                idxg[:, (CH // 16) * c:(CH // 16) * (c + 1)],
                num_idxs=CH,
                num_idxs_reg=r_vals[c],
                elem_size=DC,
                single_packet=False,
            )
            smul = sp.tile([P, FB], f32, tag="smul")
            nc.vector.tensor_tensor(smul[:], xg[:, :, D],
                                    validF[:, FB * c:FB * (c + 1)], op=AO.mult)
            for j in range(FB):
                jj = FB * c + j
                nc.vector.tensor_scalar(
                    xg[:, j, 0:D], xg[:, j, 0:D], scalar1=smul[:, j:j + 1],
                    scalar2=None, op0=AO.mult,
                )
                nc.sync.dma_start(
                    odata[P * jj:P * (jj + 1), :], xg[:, j, 0:D],
                    cond=cnt_rv > P * jj,
                )


def _build():
    _ensure_path()
    import concourse.bacc as bacc
    import concourse.mybir as mybir
    import concourse.tile as tile

    f32 = mybir.dt.float32
    i32 = mybir.dt.int32

    nc = bacc.Bacc(
        "TRN2",
        target_bir_lowering=False,
        debug=False,
        enable_asserts=True,
        num_devices=E,
    )
    ins = {
        "xcat": nc.dram_tensor("xcat", [N, DC], f32, kind="ExternalInput").ap(),
        "m16": nc.dram_tensor("m16", [16, W], i32, kind="ExternalInput").ap(),
        "scol": nc.dram_tensor("scol", [P, F], f32, kind="ExternalInput").ap(),
    }
    outs = {
        "odata": nc.dram_tensor("odata", [N, D], f32, kind="ExternalOutput").ap(),
        "opair": nc.dram_tensor("opair", [2 * N, PAIR_STRIDE], f32,
                                kind="ExternalOutput").ap(),
        "ocnt": nc.dram_tensor("ocnt", [1, 1], i32, kind="ExternalOutput").ap(),
    }
    with tile.TileContext(nc) as tc:
        _emit(tc, nc, ins, outs)
    nc.compile()
    return nc


def _get_nc():
    if "nc" not in _CACHE:
        _CACHE["nc"] = _build()
    return _CACHE["nc"]


def _install_ntff_hook():
    """Provide antenv.axon_hooks if the image lacks it (enables trace=True)."""
    try:
        from antenv.axon_hooks import get_axon_ntff_profile_hook  # noqa: F401
        return
    except ImportError:
        pass
    try:
        import types

        import antenv
        from trn_agent_boot.trn_boot import _ntff_profile_via_ctypes

        hook = _ntff_profile_via_ctypes("/opt/axon/libaxon_pjrt.so")
        mod = types.ModuleType("antenv.axon_hooks")
        mod.get_axon_ntff_profile_hook = lambda: hook
        mod.set_axon_ntff_profile_hook = lambda h: None
        sys.modules["antenv.axon_hooks"] = mod
        antenv.axon_hooks = mod
    except Exception:
        pass


def kernel(x, score, hot_mask, tag):
    _ensure_path()
    _install_ntff_hook()
    from concourse.bass_utils import run_bass_kernel_spmd

    x = np.ascontiguousarray(np.asarray(x, dtype=np.float32))
    score = np.asarray(score, dtype=np.float32)
    hot_mask = np.asarray(hot_mask, dtype=np.int32)

    nc = _get_nc()
    in_maps = []
    for e in range(E):
        xcat = np.zeros((N, DC), dtype=np.float32)
        xcat[:, :D] = x
        xcat[:, D] = score[:, e]
        in_maps.append({
            "xcat": xcat,
            "m16": np.ascontiguousarray(hot_mask[:, e].reshape(W, 16).T),
            "scol": np.ascontiguousarray(score[:, e].reshape(F, P).T),
        })
    trace = bool(int(os.environ.get("KERNEL_TRACE", "0")))
    res = run_bass_kernel_spmd(nc, in_maps, core_ids=list(range(E)), trace=trace)
    _CACHE["last_results"] = res

    out_data = np.stack([res.results[e]["odata"] for e in range(E)])
    out_tags = np.stack([res.results[e]["opair"][:N, 0:1].astype(np.int32)
                         for e in range(E)])
    counts = np.array([res.results[e]["ocnt"][0, 0] for e in range(E)],
                      dtype=np.int32)
    return out_data, out_tags, counts


# revision 35
# speedup vs baseline: 1.1028x; 1.0094x over previous
"""MoE dispatch (DispatchSF) Trainium2 Bass kernel — expert-parallel over 8 cores.

Problem: N=4096 tokens, D=1024 d_model, E=8 experts. For each expert e:
pack tokens with hot_mask[:, e] > 0 (in original order) into the first
`count` of N output slots, scaled by score[:, e]; zero-pad the rest.
One expert per NeuronCore; each core sees the full token buffer.

Device algorithm per core:
  1. exclusive prefix-sum of the mask over token order, computed directly in
     the DMA-ucode "wrapped" index layout [16, 256] (token i at [i%16, i//16]):
     within-column via a [16,16] strict-upper-triangular PE matmul, across
     columns via a [1, 256] tensor_tensor_scan + K=1 broadcast matmul.
  2. compaction: ONE dma_scatter_add ucode op scatters (token_as_f32, score)
     pairs into a 256B-strided [2N, 64] f32 output buffer (opair) at row
     `excl_prefix` for selected tokens, row `N + token` (dump) otherwise.
     opair arrives zero-filled (donated output buffer), so add == write and
     the packed zone's tail stays (0.0, 0.0).
  3. rebuild the packed token list as a wrapped int16 index table
     (load-back + two PE transposes), pad it with -1 beyond
     ceil(count/128)*128 slots (chunk heads kept valid).
  4. 4 chunked dma_gather ucode ops fetch the selected rows of xcat
     (= x row ++ score ++ pad, 1088 f32) — only ~count rows are read.
     Per 128-slot block: DVE scale by (score lane x slot-validity), then a
     conditional store; blocks past `count` are skipped entirely and stay
     zero via the donated zero output buffer.

Host slices out_tags from opair[:, :N, 0] and counts from ocnt.
"""

import os
import sys

import numpy as np

N, D, E = 4096, 1024, 8
P = 128
F = N // P          # 32 blocks of 128 slots
W = N // 16         # 256 wrapped columns
DC = D + 64         # concat row: x ++ score ++ pad (4352 B, 17*256)
NCHUNK = 4
CH = N // NCHUNK    # 1024 slots per gather chunk
PAIR_STRIDE = 64    # f32 elements per opair row (256 B stride requirement)

_CACHE = {}


def _ensure_path():
    for p in ("/opt/trn_rl_repo",):
        if p not in sys.path:
            sys.path.insert(0, p)


def _emit(tc, nc, ins, outs):
    """Emit the per-core device program. ins/outs: dicts of DRAM APs."""
    import concourse.mybir as mybir
    from concourse.masks import make_identity, make_upper_triangular

    f32 = mybir.dt.float32
    i32 = mybir.dt.int32
    i16 = mybir.dt.int16
    AO = mybir.AluOpType

    xcat = ins["xcat"]    # [N, DC] f32: row = x[i] ++ score[i] ++ zeros
    m16 = ins["m16"]      # [16, W] i32 wrapped mask: [q, s] = mask[s*16+q]
    scol = ins["scol"]    # [P, F] f32: [p, f] = score[f*128 + p]
    odata = outs["odata"]  # [N, D] f32
    opair = outs["opair"]  # [2N, 64] f32; row s cols 0:2 = (token, score)
    ocnt = outs["ocnt"]    # [1, 1] i32

    with (
        tc.tile_pool(name="small", bufs=1) as sp,
        tc.tile_pool(name="psum", bufs=1, space="PSUM") as pp,
        tc.tile_pool(name="xc", bufs=NCHUNK) as xcp,
    ):
        # ---------- constants ----------
        utri16 = sp.tile([16, 16], f32)
        make_upper_triangular(nc, utri16[:], val=1.0, diag=False)
        ident = sp.tile([P, P], f32)
        make_identity(nc, ident[:])
        ones16c = sp.tile([16, 1], f32)
        nc.vector.memset(ones16c[:], 1.0)
        ones16r = sp.tile([1, 16], f32)
        nc.vector.memset(ones16r[:], 1.0)
        ones128r = sp.tile([1, P], f32)
        nc.vector.memset(ones128r[:], 1.0)

        tok16I = sp.tile([16, W], i32)   # token id q + 16s
        nc.gpsimd.iota(tok16I[:], pattern=[[16, W]], base=0, channel_multiplier=1)
        tok16F = sp.tile([16, W], f32)
        nc.vector.tensor_copy(tok16F[:], tok16I[:])
        tok128I = sp.tile([P, F], i32)   # token/slot id p + 128f
        nc.gpsimd.iota(tok128I[:], pattern=[[P, F]], base=0, channel_multiplier=1)
        tok128F = sp.tile([P, F], f32)
        nc.vector.tensor_copy(tok128F[:], tok128I[:])
        blkI = sp.tile([1, F], i32)      # 128*j block starts
        nc.gpsimd.iota(blkI[:], pattern=[[P, F]], base=0, channel_multiplier=0)
        blkF = sp.tile([1, F], f32)
        nc.vector.tensor_copy(blkF[:], blkI[:])
        chI = sp.tile([1, NCHUNK], i32)  # 1024*c chunk starts
        nc.gpsimd.iota(chI[:], pattern=[[CH, NCHUNK]], base=0, channel_multiplier=0)
        chF = sp.tile([1, NCHUNK], f32)
        nc.vector.tensor_copy(chF[:], chI[:])

        # ---------- exclusive prefix over token order (wrapped layout) ----------
        m16I = sp.tile([16, W], i32)
        nc.sync.dma_start(m16I[:], m16)
        m16F = sp.tile([16, W], f32)
        nc.vector.tensor_copy(m16F[:], m16I[:])

        excl_ps = pp.tile([16, W], f32, space="PSUM")
        nc.tensor.matmul(excl_ps[:], lhsT=utri16[:], rhs=m16F[:],
                         start=True, stop=True)
        colsum_ps = pp.tile([1, W], f32, space="PSUM")
        nc.tensor.matmul(colsum_ps[:], lhsT=ones16c[:], rhs=m16F[:],
                         start=True, stop=True)
        # scan directly off PSUM (data1 is ignored under bypass but must be
        # a non-PSUM operand; hm is a ready zero-filled SBUF tile)
        hm = sp.tile([1, W], f32)
        nc.vector.memset(hm[:], 0.0)
        for c in range(NCHUNK):
            nc.vector.memset(hm[:, (CH // 16) * c:(CH // 16) * c + 1], 1.0)
        colsumS = sp.tile([1, W], f32)
        nc.vector.tensor_copy(colsumS[:], colsum_ps[:])
        colincl = sp.tile([1, W], f32)
        nc.vector.tensor_tensor_scan(
            colincl[:], colsumS[:], hm[:], initial=0.0,
            op0=AO.add, op1=AO.bypass,
        )
        colexcl = sp.tile([1, W], f32)
        nc.vector.tensor_tensor(colexcl[:], colincl[:], colsumS[:],
                                op=AO.subtract)
        coloff_ps = pp.tile([16, W], f32, space="PSUM")
        nc.tensor.matmul(coloff_ps[:], lhsT=ones16r[:], rhs=colexcl[:],
                         start=True, stop=True)
        coloffS = sp.tile([16, W], f32)
        nc.vector.tensor_copy(coloffS[:], coloff_ps[:])
        excl16 = sp.tile([16, W], f32)
        nc.vector.tensor_tensor(excl16[:], excl_ps[:], coloffS[:], op=AO.add)

        # count
        cntF = sp.tile([1, 1], f32)
        nc.vector.tensor_copy(cntF[:], colincl[:, W - 1:W])
        cntI = sp.tile([1, 1], i32)
        nc.vector.tensor_copy(cntI[:], cntF[:])
        nc.sync.dma_start(ocnt[:], cntI[:])

        # scatter dest = m ? excl : N + token  (unique rows in [0, 2N))
        # d1 = (excl - N) - tok ; d2 = d1 * m ; dest = (d2 + N) + tok
        d1 = sp.tile([16, W], f32)
        nc.vector.scalar_tensor_tensor(d1[:], excl16[:], -float(N), tok16F[:],
                                       op0=AO.add, op1=AO.subtract)
        d2 = sp.tile([16, W], f32)
        nc.vector.tensor_tensor(d2[:], d1[:], m16F[:], op=AO.mult)
        destW = sp.tile([16, W], f32)
        nc.vector.scalar_tensor_tensor(destW[:], d2[:], float(N), tok16F[:],
                                       op0=AO.add, op1=AO.add)

        # wrapped scatter idx table, int16, replicated to 128 partitions
        idx16 = sp.tile([P, W], i16)
        nc.vector.tensor_copy(idx16[0:16, :], destW[:])
        nc.sync.dma_start(idx16[16:32, :], idx16[0:16, :])
        nc.vector.tensor_copy(idx16[32:64, :], idx16[0:32, :])
        nc.vector.tensor_copy(idx16[64:128, :], idx16[0:64, :])

        # ---------- compaction scatter ----------
        # payload (token, score) for token i at [i%128, i//128]
        sF = sp.tile([P, F], f32)
        nc.sync.dma_start(sF[:], scol)
        pairV = sp.tile([P, 2 * F], f32)
        pair2 = pairV[:].rearrange("p (f c) -> p f c", c=2)
        nc.vector.tensor_copy(pair2[:, :, 0], tok128F[:])
        nc.vector.tensor_copy(pair2[:, :, 1], sF[:])

        nc.gpsimd.dma_scatter_add(
            out_ap=opair[:, 0:2],
            in_ap=pair2[:, :, :],
            idxs_ap=idx16[:],
            num_idxs=N,
            num_idxs_reg=N,
            elem_size=2,
            elem_step=PAIR_STRIDE,
            single_packet=False,
        )

        # ---------- rebuild packed tokens as wrapped gather idx ----------
        # load the whole packed zone contiguously (1 MiB, 128 fat descriptors);
        # tokens of slot s = 32p + t sit at ldFull[p, 64t]
        ldFull = sp.tile([P, N * PAIR_STRIDE // P], f32)
        nc.sync.dma_start(
            ldFull[:], opair[0:N, :].rearrange("(p t) c -> p (t c)", p=P),
        )
        ld3 = ldFull[:].rearrange("p (t c) -> p t c", c=PAIR_STRIDE)
        ldT = sp.tile([P, F], f32)
        nc.vector.tensor_copy(ldT[:], ld3[:, :, 0])
        # wrapped[q, 2p+u] = token of slot 32p + 16u + q -> two PE transposes
        t0_ps = pp.tile([16, P], f32, space="PSUM")
        nc.tensor.transpose(t0_ps[:], ldT[:, 0:16], ident[:])
        t1_ps = pp.tile([16, P], f32, space="PSUM")
        nc.tensor.transpose(t1_ps[:], ldT[:, 16:32], ident[:])

        # block validity bv[j] = (128j < count), expanded to wrapped columns
        bv = sp.tile([1, F], f32)
        nc.vector.tensor_scalar(bv[:], blkF[:], scalar1=cntF[:, :1],
                                scalar2=None, op0=AO.is_lt)
        bv256 = sp.tile([1, W], f32)
        bv3 = bv256[:].rearrange("p (a k) -> p a k", k=8)
        nc.vector.tensor_copy(bv3[:, :, :],
                              bv[:, :, None].broadcast_to([1, F, 8]))
        # hm (chunk-head columns = 1) keeps every dma_gather chunk non-empty
        bvh = sp.tile([1, W], f32)
        nc.vector.tensor_tensor(bvh[:], bv256[:], hm[:], op=AO.max)
        bvh16_ps = pp.tile([16, W], f32, space="PSUM")
        nc.tensor.matmul(bvh16_ps[:], lhsT=ones16r[:], rhs=bvh[:],
                         start=True, stop=True)

        # idw[q, s] = packed token of slot 16s+q; adj = (idw+1)*bvh - 1
        idw = sp.tile([16, W], f32)
        idw3 = idw[:].rearrange("q (p u) -> q p u", u=2)
        nc.vector.tensor_copy(idw3[:, :, 0], t0_ps[:])
        nc.vector.tensor_copy(idw3[:, :, 1], t1_ps[:])
        adjF = sp.tile([16, W], f32)
        nc.vector.scalar_tensor_tensor(adjF[:], idw[:], 1.0, bvh16_ps[:],
                                       op0=AO.add, op1=AO.mult)
        idxg = sp.tile([P, W], i16)
        nc.vector.tensor_scalar(idxg[0:16, :], adjF[:], scalar1=1.0,
                                scalar2=None, op0=AO.subtract)
        nc.sync.dma_start(idxg[16:32, :], idxg[0:16, :])
        nc.vector.tensor_copy(idxg[32:64, :], idxg[0:32, :])
        nc.vector.tensor_copy(idxg[64:128, :], idxg[0:64, :])

        # ---------- runtime registers ----------
        # per-chunk gather counts r_c = clamp(128*sum(bv) - 1024c, 1, 1024)
        nb = sp.tile([1, 1], f32)
        nc.vector.tensor_reduce(nb[:], bv[:], axis=mybir.AxisListType.X,
                                op=AO.add)
        cnt128F = sp.tile([1, 1], f32)
        nc.vector.tensor_scalar(cnt128F[:], nb[:], scalar1=float(P),
                                scalar2=None, op0=AO.mult)
        r4 = sp.tile([1, NCHUNK], f32)
        nc.vector.tensor_scalar(r4[:], chF[:], scalar1=cnt128F[:, :1],
                                scalar2=-1.0, op0=AO.subtract, op1=AO.mult)
        nc.vector.tensor_scalar(r4[:], r4[:], scalar1=16.0, scalar2=float(CH),
                                op0=AO.max, op1=AO.min)
        r4I = sp.tile([1, NCHUNK], i32)
        nc.vector.tensor_copy(r4I[:], r4[:])
        _, r_vals = nc.values_load_multi_w_load_instructions(
            r4I[:], min_val=1, max_val=CH, skip_runtime_bounds_check=True)
        cnt_rv = nc.values_load(cntI[:], min_val=0, max_val=N,
                                skip_runtime_bounds_check=True)

        # slot validity for the straddle block: valid[p, f] = (p + 128f < count)
        cntB_ps = pp.tile([P, 1], f32, space="PSUM")
        nc.tensor.matmul(cntB_ps[:], lhsT=ones128r[:], rhs=cntF[:],
                         start=True, stop=True)
        cntBS = sp.tile([P, 1], f32)
        nc.vector.tensor_copy(cntBS[:], cntB_ps[:])
        validF = sp.tile([P, F], f32)
        nc.vector.tensor_scalar(validF[:], tok128F[:], scalar1=cntBS[:, :1],
                                scalar2=None, op0=AO.is_lt)

        # ---------- gather chunks, scale, conditional store ----------
        FB = F // NCHUNK  # blocks per chunk (8)
        for c in range(NCHUNK):
            xg = xcp.tile([P, FB, DC], f32, tag="xc")
            nc.gpsimd.dma_gather(
                xg[:, :, :],
                xcat,
                idxg[:, (CH // 16) * c:(CH // 16) * (c + 1)],
                num_idxs=CH,
                num_idxs_reg=r_vals[c],
                elem_size=DC,
                single_packet=False,
            )
            smul = sp.tile([P, FB], f32, tag="smul")
            nc.vector.tensor_tensor(smul[:], xg[:, :, D],
                                    validF[:, FB * c:FB * (c + 1)], op=AO.mult)
            for j in range(FB):
                jj = FB * c + j
                nc.vector.tensor_scalar(
                    xg[:, j, 0:D], xg[:, j, 0:D], scalar1=smul[:, j:j + 1],
                    scalar2=None, op0=AO.mult,
                )
                nc.sync.dma_start(
                    odata[P * jj:P * (jj + 1), :], xg[:, j, 0:D],
                    cond=cnt_rv > P * jj,
                )


def _build():
    _ensure_path()
    import concourse.bacc as bacc
    import concourse.mybir as mybir
    import concourse.tile as tile

    f32 = mybir.dt.float32
    i32 = mybir.dt.int32

    nc = bacc.Bacc(
        "TRN2",
        target_bir_lowering=False,
        debug=False,
        enable_asserts=True,
        num_devices=E,
    )
    ins = {
        "xcat": nc.dram_tensor("xcat", [N, DC], f32, kind="ExternalInput").ap(),
        "m16": nc.dram_tensor("m16", [16, W], i32, kind="ExternalInput").ap(),
        "scol": nc.dram_tensor("scol", [P, F], f32, kind="ExternalInput").ap(),
    }
    outs = {
        "odata": nc.dram_tensor("odata", [N, D], f32, kind="ExternalOutput").ap(),
        "opair": nc.dram_tensor("opair", [2 * N, PAIR_STRIDE], f32,
                                kind="ExternalOutput").ap(),
        "ocnt": nc.dram_tensor("ocnt", [1, 1], i32, kind="ExternalOutput").ap(),
    }
    with tile.TileContext(nc) as tc:
        _emit(tc, nc, ins, outs)
    nc.compile()
    return nc


def _get_nc():
    if "nc" not in _CACHE:
        _CACHE["nc"] = _build()
    return _CACHE["nc"]


def _install_ntff_hook():
    """Provide antenv.axon_hooks if the image lacks it (enables trace=True)."""
    try:
        from antenv.axon_hooks import get_axon_ntff_profile_hook  # noqa: F401
        return
    except ImportError:
        pass
    try:
        import types

        import antenv
        from trn_agent_boot.trn_boot import _ntff_profile_via_ctypes

        hook = _ntff_profile_via_ctypes("/opt/axon/libaxon_pjrt.so")
        mod = types.ModuleType("antenv.axon_hooks")
        mod.get_axon_ntff_profile_hook = lambda: hook
        mod.set_axon_ntff_profile_hook = lambda h: None
        sys.modules["antenv.axon_hooks"] = mod
        antenv.axon_hooks = mod
    except Exception:
        pass


def kernel(x, score, hot_mask, tag):
    _ensure_path()
    _install_ntff_hook()
    from concourse.bass_utils import run_bass_kernel_spmd

    x = np.ascontiguousarray(np.asarray(x, dtype=np.float32))
    score = np.asarray(score, dtype=np.float32)
    hot_mask = np.asarray(hot_mask, dtype=np.int32)

    nc = _get_nc()
    in_maps = []
    for e in range(E):
        xcat = np.zeros((N, DC), dtype=np.float32)
        xcat[:, :D] = x
        xcat[:, D] = score[:, e]
        in_maps.append({
            "xcat": xcat,
            "m16": np.ascontiguousarray(hot_mask[:, e].reshape(W, 16).T),
            "scol": np.ascontiguousarray(score[:, e].reshape(F, P).T),
        })
    trace = bool(int(os.environ.get("KERNEL_TRACE", "0")))
    res = run_bass_kernel_spmd(nc, in_maps, core_ids=list(range(E)), trace=trace)
    _CACHE["last_results"] = res

    out_data = np.stack([res.results[e]["odata"] for e in range(E)])
    out_tags = np.stack([res.results[e]["opair"][:N, 0:1].astype(np.int32)
                         for e in range(E)])
    counts = np.array([res.results[e]["ocnt"][0, 0] for e in range(E)],
                      dtype=np.int32)
    return out_data, out_tags, counts
